# revision 5
# baseline (speedup 1.0000x reference)
"""Trainium2 Bass kernel for nn_Attention (B=4, S=2048, HIDDEN=768, 12 heads).

Sharding: 8 cores = 4 batches x 2 head-groups (6 heads each). Projection
weights are sliced per head-group and pre-transposed on the host; the
1/sqrt(64) scale is folded into Wq. Each core computes a partial output
(its head-group's contribution through Wo, with bo/2 bias); the host sums
the two partials per batch.

v2 design notes:
- All projection operands in bf16 (FWL-eligible weight loads; 128-col
  stationaries). Attention matmuls bf16 with fp32 PSUM accumulate.
- Scores for a head pair are issued interleaved with explicit
  tile_position (0,0)/(64,0) so the two 64-contraction matmuls run
  concurrently in separate PE row-groups.
- exp() is split: most tiles on ScalarE (exact spline), a fraction on
  VectorE via a double-Schraudolph bit-trick (two fp32->int16 affine
  converts bitcast as bf16, multiplied; ~2% ripple, scale cancels in
  softmax). This balances the two engines; ScalarE alone would be the
  bottleneck at ~210us.
- Softmax denominator rides the attention matmul as an extra all-ones
  v-column (row 64 of the 65-row accumulator). PSUM is evacuated with a
  single [65, QT] copy (frees the PSUM slot fast); denominators of a head
  pair round-trip through DRAM to pack 2x1024 values as [128, 16], one
  cheap approx-reciprocal, then DMA-broadcast per head for the normalize
  multiply (SBUF -> bf16 attn).
- PE is warmed with dummy matmuls from t~=0 so the HAM clock-gate opens
  before the projection phase instead of 50us into it.
"""

import numpy as np
import ml_dtypes

HIDDEN = 768
NUM_HEADS = 12
D = 64
B = 4
S = 2048
P = 128

H_CORE = 6          # heads per core
OC = H_CORE * D     # 384 output channels per core for q/k
WAUG = H_CORE * (D + 1)  # 390: v columns with interleaved ones-columns
C_CHUNKS = HIDDEN // P   # 6
QT = 1024           # q-tile (free dim) for the attention inner loop
MMF = 512           # matmul moving free dim
STILES = S // P     # 16
QTILES = OC // P    # 3
NQH = S // QT       # 2
NF = QT // MMF      # 2

# double-Schraudolph exp approximation constants (bf16 int16 bit trick)
EXP_A = float(np.log2(np.e) * 64.0)
EXP_B1 = float(127 * 128 - 38.3)
EXP_B2 = float(127 * 128 + 24.2)

_CACHE = {}


def _build(with_mask: bool):
    import concourse.bass as bass
    import concourse.tile as tile
    from concourse import bacc, mybir
    from contextlib import ExitStack

    f32 = mybir.dt.float32
    bf16 = mybir.dt.bfloat16
    i16 = mybir.dt.int16
    AF = mybir.ActivationFunctionType
    ALU = mybir.AluOpType

    nc = bacc.Bacc(
        "TRN2",
        target_bir_lowering=False,
        debug=False,
        enable_asserts=True,
        num_devices=8,
    )

    x_d = nc.dram_tensor("x", (HIDDEN, S), bf16, kind="ExternalInput").ap()
    wq_d = nc.dram_tensor("wqT", (HIDDEN, OC), bf16, kind="ExternalInput").ap()
    bq_d = nc.dram_tensor("bq", (OC,), f32, kind="ExternalInput").ap()
    wk_d = nc.dram_tensor("wkT", (HIDDEN, OC), bf16, kind="ExternalInput").ap()
    bk_d = nc.dram_tensor("bk", (OC,), f32, kind="ExternalInput").ap()
    wv_d = nc.dram_tensor("wvT", (HIDDEN, WAUG), bf16, kind="ExternalInput").ap()
    bvb_d = nc.dram_tensor("bvb", (P, WAUG), f32, kind="ExternalInput").ap()
    wo_d = nc.dram_tensor("woT", (OC, HIDDEN), bf16, kind="ExternalInput").ap()
    bo_d = nc.dram_tensor("bo", (HIDDEN,), f32, kind="ExternalInput").ap()
    if with_mask:
        em_d = nc.dram_tensor("expmask", (S, S), f32, kind="ExternalInput").ap()
    out_d = nc.dram_tensor("out", (HIDDEN, S), f32, kind="ExternalOutput").ap()

    x_r = x_d.rearrange("(t p) s -> p t s", p=P)
    wq_r = wq_d.rearrange("(t p) o -> p t o", p=P)
    wk_r = wk_d.rearrange("(t p) o -> p t o", p=P)
    wv_r = wv_d.rearrange("(t p) o -> p t o", p=P)
    wo_r = wo_d.rearrange("(t p) o -> p t o", p=P)
    bq_r = bq_d.rearrange("(t p) -> p t", p=P)
    bk_r = bk_d.rearrange("(t p) -> p t", p=P)
    bo_r = bo_d.rearrange("(t p) -> p t", p=P)
    out_r = out_d.rearrange("(t p) s -> p t s", p=P)

    with tile.TileContext(nc) as tc, ExitStack() as ctx:
        consts = ctx.enter_context(tc.tile_pool(name="consts", bufs=1))
        persist = ctx.enter_context(tc.tile_pool(name="persist", bufs=1))

        bq_t = consts.tile([P, QTILES], f32)
        nc.sync.dma_start(bq_t[:], bq_r)
        bk_t = consts.tile([P, QTILES], f32)
        nc.sync.dma_start(bk_t[:], bk_r)
        bo_t = consts.tile([P, C_CHUNKS], f32)
        nc.sync.dma_start(bo_t[:], bo_r)
        bvb_t = consts.tile([P, WAUG], f32)
        nc.sync.dma_start(bvb_t[:], bvb_d)
        wo_t = consts.tile([P, QTILES, HIDDEN], bf16)
        nc.sync.dma_start(wo_t[:], wo_r)

        q_t = persist.tile([P, QTILES, S], bf16)
        k_t = persist.tile([P, QTILES, S], bf16)
        v_t = persist.tile([P, STILES, WAUG], bf16)
        attn_t = persist.tile([P, QTILES, S], bf16)

        # ---------------- phase A: projections ----------------
        with (
            tc.tile_pool(name="phA", bufs=1) as phA,
            tc.tile_pool(name="psA", bufs=2, space="PSUM") as psA,
        ):
            # HAM warmup from t=0: PE idles during the input DMA, and cold
            # matmuls run at 1.2 GHz. A continuous dummy-matmul stream from
            # kernel start keeps the PE busy through the HAM window so the
            # real projections start at 2.4 GHz.
            wut = phA.tile([P, MMF], bf16, name="wu_src")
            nc.vector.memset(wut[:], 0.0)
            wu = psA.tile([P, MMF], f32, tag="Pv", name="warmA")
            for i in range(44):
                nc.tensor.matmul(
                    wu[:], wut[:, 0:P], wut[:], start=True, stop=True,
                )

            x_t = phA.tile([P, C_CHUNKS, S], bf16)
            wq_t = phA.tile([P, C_CHUNKS, OC], bf16)
            wk_t = phA.tile([P, C_CHUNKS, OC], bf16)
            wv_t = phA.tile([P, C_CHUNKS, WAUG], bf16)
            for c in range(C_CHUNKS):
                nc.sync.dma_start(wq_t[:, c, :], wq_r[:, c, :])
                # split x chunk across two queues to halve its latency
                nc.sync.dma_start(x_t[:, c, 0:S // 2], x_r[:, c, 0:S // 2])
                nc.sync.dma_start(x_t[:, c, S // 2:S], x_r[:, c, S // 2:S])
                nc.sync.dma_start(wk_t[:, c, :], wk_r[:, c, :])
                nc.sync.dma_start(wv_t[:, c, :], wv_r[:, c, :])

            # q, k projections: out[o_tile(128), s] = sum_c WT[c,o]^T x[c,s]
            for ot in range(QTILES):
                for di, (dst, w_sb, b_sb) in enumerate(
                    ((q_t, wq_t, bq_t), (k_t, wk_t, bk_t))
                ):
                    for half in range(S // QT):
                        ps = psA.tile([P, QT], f32, tag="Pq")
                        for c in range(C_CHUNKS):
                            for nf in range(NF):
                                nc.tensor.matmul(
                                    ps[:, nf * MMF:(nf + 1) * MMF],
                                    w_sb[:, c, ot * P:(ot + 1) * P],
                                    x_t[:, c, half * QT + nf * MMF:
                                        half * QT + (nf + 1) * MMF],
                                    start=(c == 0),
                                    stop=(c == C_CHUNKS - 1),
                                )
                        # alternate evacuation engine: DVE and ScalarE both
                        # read PSUM; splitting halves the critical path
                        if (ot * 4 + di * 2 + half) % 2 == 0:
                            nc.vector.tensor_scalar_add(
                                dst[:, ot, half * QT:(half + 1) * QT],
                                ps[:],
                                b_sb[:, ot:ot + 1],
                            )
                        else:
                            nc.scalar.activation(
                                dst[:, ot, half * QT:(half + 1) * QT],
                                ps[:],
                                AF.Identity,
                                bias=b_sb[:, ot:ot + 1],
                            )

            # vT projection: out[s_tile(128), 390] = sum_c x[c,s]^T WvT[c,:]
            for st in range(STILES):
                ps = psA.tile([P, WAUG], f32, tag="Pv")
                for c in range(C_CHUNKS):
                    nc.tensor.matmul(
                        ps[:],
                        x_t[:, c, st * P:(st + 1) * P],
                        wv_t[:, c, :],
                        start=(c == 0),
                        stop=(c == C_CHUNKS - 1),
                    )
                nc.vector.tensor_tensor(
                    v_t[:, st, :], ps[:], bvb_t[:], ALU.add
                )

        # ---------------- phase B: attention ----------------
        with (
            tc.tile_pool(name="phB", bufs=4) as phB,
            tc.tile_pool(name="psB", bufs=2, space="PSUM") as psB,
            tc.tile_pool(name="outp", bufs=2) as outp,
            tc.tile_pool(name="dscr", bufs=2, space="DRAM") as dscr,
        ):
            # HAM warmup: the proj->attention boundary idles the PE long
            # enough to clock-gate it to 1.2 GHz, and the attention phase's
            # short busy bursts never un-throttle it on their own. A ~6us
            # dense burst of dummy matmuls flips it back to 2.4 GHz; the
            # steady-state attention gaps then never re-throttle it.
            wu = psB.tile([P, QT], f32, tag="S", name="warmup")
            for i in range(16):
                nc.tensor.matmul(
                    wu[:, 0:MMF], wo_t[:, 0, 0:P], wo_t[:, 0, 0:MMF],
                    start=True, stop=True,
                )
            for qh in range(NQH):
                for hp in range(H_CORE // 2):
                    heads = (2 * hp, 2 * hp + 1)
                    accs = [
                        psB.tile([D + 1, QT], f32, tag="A", name=f"acc{i}")
                        for i in range(2)
                    ]
                    for c in range(STILES):
                        scs = [
                            psB.tile([P, QT], f32, tag="S", name=f"sc{i}")
                            for i in range(2)
                        ]
                        # head-pair scores interleaved: rows 0-63 and 64-127
                        # of the PE run concurrently (explicit tile_position)
                        for nf in range(NF):
                            for hi in range(2):
                                pb = 64 * hi
                                nc.tensor.matmul(
                                    scs[hi][:, nf * MMF:(nf + 1) * MMF],
                                    k_t[pb:pb + D, hp, c * P:(c + 1) * P],
                                    q_t[pb:pb + D, hp,
                                        qh * QT + nf * MMF:
                                        qh * QT + (nf + 1) * MMF],
                                    start=True,
                                    stop=True,
                                    tile_position=(pb, 0),
                                )
                        etiles = []
                        for hi in range(2):
                            e = phB.tile([P, QT], bf16, tag="E")
                            # ~25% of tiles take the approximate DVE path to
                            # offload the ScalarE exp bottleneck
                            if hi == 1 and c % 2 == 0:
                                y1 = phB.tile([P, QT], i16, tag="Y")
                                y2 = phB.tile([P, QT], i16, tag="Y")
                                nc.vector.tensor_scalar(
                                    y1[:], scs[hi][:], EXP_A, EXP_B1,
                                    ALU.mult, ALU.add,
                                )
                                nc.vector.tensor_scalar(
                                    y2[:], scs[hi][:], EXP_A, EXP_B2,
                                    ALU.mult, ALU.add,
                                )
                                nc.vector.tensor_tensor(
                                    e[:], y1[:].bitcast(bf16),
                                    y2[:].bitcast(bf16), ALU.mult,
                                )
                            else:
                                nc.scalar.activation(e[:], scs[hi][:], AF.Exp)
                            if with_mask:
                                em = phB.tile([P, QT], f32, tag="M")
                                nc.sync.dma_start(
                                    em[:],
                                    em_d[c * P:(c + 1) * P,
                                         qh * QT:(qh + 1) * QT],
                                )
                                nc.vector.tensor_tensor(
                                    e[:], e[:], em[:], ALU.mult
                                )
                            etiles.append(e)
                        for hi, h in enumerate(heads):
                            for nf in range(NF):
                                nc.tensor.matmul(
                                    accs[hi][:, nf * MMF:(nf + 1) * MMF],
                                    v_t[:, c, 65 * h:65 * h + 65],
                                    etiles[hi][:, nf * MMF:(nf + 1) * MMF],
                                    start=(c == 0),
                                    stop=(c == STILES - 1),
                                )

                    # ---- per-pair softmax normalization ----
                    # single [65, QT] copy evacuates numerators + denominator
                    # (frees the PSUM slot); denominators round-trip through
                    # DRAM packed as [128, 16] for one cheap reciprocal.
                    us = []
                    scr = dscr.tile([2, QT], f32, name=f"scr{qh}_{hp}")
                    for hi, h in enumerate(heads):
                        u = phB.tile([D + 1, QT], f32, tag="U", bufs=5,
                                     name=f"u{qh}_{h}")
                        nc.vector.tensor_copy(u[:], accs[hi][:])
                        nc.sync.dma_start(scr[hi:hi + 1, :], u[D:D + 1, :])
                        us.append(u)
                    pk = phB.tile([P, 2 * QT // P], f32, tag="PK", bufs=2,
                                  name=f"pk{qh}_{hp}")
                    nc.sync.dma_start(
                        pk[:], scr.rearrange("a (p n) -> (a p) n", p=P // 2)
                    )
                    rec = phB.tile([P, 2 * QT // P], f32, tag="PK", bufs=2,
                                   name=f"rec{qh}_{hp}")
                    nc.vector.reciprocal_approx_fast(rec[:], pk[:])
                    scr2 = dscr.tile([2, QT], f32, name=f"scr2_{qh}_{hp}")
                    nc.sync.dma_start(
                        scr2.rearrange("a (p n) -> (a p) n", p=P // 2), rec[:]
                    )
                    for hi, h in enumerate(heads):
                        bc = phB.tile([D, QT], f32, tag="B")
                        nc.sync.dma_start(
                            bc[:], scr2[hi:hi + 1, :].to_broadcast((D, QT))
                        )
                        pb = 64 * hi
                        nc.vector.tensor_tensor(
                            attn_t[pb:pb + D, hp, qh * QT:(qh + 1) * QT],
                            us[hi][0:D, :],
                            bc[:],
                            ALU.mult,
                        )

            # ---------------- output projection (tail) ----------------
            # re-warm the PE after the final normalization-chain gap
            wu2 = psB.tile([P, QT], f32, tag="S", name="warmup2")
            for i in range(12):
                nc.tensor.matmul(
                    wu2[:, 0:MMF], wo_t[:, 0, 0:P], wo_t[:, 0, 0:MMF],
                    start=True, stop=True,
                )
            for qh in range(NQH):
                for ot in range(C_CHUNKS):
                    ps = psB.tile([P, QT], f32, tag="S")
                    for ct in range(QTILES):
                        for nf in range(NF):
                            nc.tensor.matmul(
                                ps[:, nf * MMF:(nf + 1) * MMF],
                                wo_t[:, ct, ot * P:(ot + 1) * P],
                                attn_t[:, ct, qh * QT + nf * MMF:
                                       qh * QT + (nf + 1) * MMF],
                                start=(ct == 0),
                                stop=(ct == QTILES - 1),
                            )
                    o_sb = outp.tile([P, QT], f32, tag="O")
                    if ot % 2 == 0:
                        nc.vector.tensor_scalar_add(
                            o_sb[:], ps[:], bo_t[:, ot:ot + 1]
                        )
                    else:
                        nc.scalar.activation(
                            o_sb[:], ps[:], AF.Identity,
                            bias=bo_t[:, ot:ot + 1],
                        )
                    nc.sync.dma_start(
                        out_r[:, ot, qh * QT:(qh + 1) * QT], o_sb[:]
                    )

    nc.compile()
    return nc


def _get_program(with_mask: bool):
    key = ("prog", with_mask)
    if key not in _CACHE:
        _CACHE[key] = _build(with_mask)
    return _CACHE[key]


def _prep_inputs(hidden_state, mask, Wq, bq, Wk, bk, Wv, bv, Wo, bo):
    """Build the 8 per-core input dicts (host-side shard + weight prep)."""
    f = np.float32
    bf = ml_dtypes.bfloat16
    scale = np.float32(D ** -0.5)
    with_mask = bool(np.any(mask))

    in_maps = []
    for b in range(B):
        x_b = np.asarray(hidden_state[b, :, 0, :], dtype=f).astype(bf)
        if with_mask:
            em_b = np.exp(mask[b, :, 0, :].astype(f))
        for g in range(2):
            rows = slice(OC * g, OC * (g + 1))
            wqT = (np.asarray(Wq[rows, :], dtype=f) * scale).T.astype(bf)
            bqs = np.ascontiguousarray(np.asarray(bq[rows], dtype=f) * scale)
            wkT = np.asarray(Wk[rows, :], dtype=f).T.astype(bf)
            bks = np.ascontiguousarray(bk[rows], dtype=f)
            # augmented v weights: col 65h+j = Wv row, col 65h+64 = 0 (bias 1)
            wvT = np.zeros((HIDDEN, WAUG), dtype=f)
            bvb = np.zeros((WAUG,), dtype=f)
            for h in range(H_CORE):
                wvT[:, 65 * h:65 * h + 64] = np.asarray(
                    Wv[OC * g + D * h:OC * g + D * h + D, :], dtype=f).T
                bvb[65 * h:65 * h + 64] = bv[OC * g + D * h:OC * g + D * h + D]
                bvb[65 * h + 64] = 1.0
            woT = np.asarray(Wo[:, rows], dtype=f).T.astype(bf)
            m = {
                "x": np.ascontiguousarray(x_b),
                "wqT": np.ascontiguousarray(wqT),
                "bq": bqs,
                "wkT": np.ascontiguousarray(wkT),
                "bk": bks,
                "wvT": wvT.astype(bf),
                "bvb": np.broadcast_to(bvb, (P, WAUG)).copy(),
                "woT": np.ascontiguousarray(woT),
                "bo": (np.asarray(bo, dtype=f) * np.float32(0.5)),
            }
            if with_mask:
                m["expmask"] = em_b
            in_maps.append(m)
    return in_maps, with_mask


def run(inputs: dict, trace: bool = False):
    """Run on 8 NeuronCores; returns (full_output, exec_time_ns_or_None)."""
    from concourse import bass_utils

    in_maps, with_mask = _prep_inputs(**inputs)
    nc = _get_program(with_mask)
    res = bass_utils.run_bass_kernel_spmd(
        nc, in_maps, core_ids=list(range(8)), trace=trace
    )
    out = np.empty((B, HIDDEN, 1, S), dtype=np.float32)
    for b in range(B):
        out[b, :, 0, :] = res.results[2 * b]["out"] + res.results[2 * b + 1]["out"]
    return out, res.exec_time_ns


def kernel(**inputs) -> np.ndarray:
    out, _ = run(inputs, trace=False)
    return out


# revision 6
# speedup vs baseline: 1.0387x; 1.0387x over previous
"""Trainium2 Bass kernel for nn_Attention (B=4, S=2048, HIDDEN=768, 12 heads).

Sharding: 8 cores = 4 batches x 2 head-groups (6 heads each). Projection
weights are sliced per head-group and pre-transposed on the host; the
1/sqrt(64) scale is folded into Wq. Each core computes a partial output
(its head-group's contribution through Wo, with bo/2 bias); the host sums
the two partials per batch.

v2 design notes:
- All projection operands in bf16 (FWL-eligible weight loads; 128-col
  stationaries). Attention matmuls bf16 with fp32 PSUM accumulate.
- Scores for a head pair are issued interleaved with explicit
  tile_position (0,0)/(64,0) so the two 64-contraction matmuls run
  concurrently in separate PE row-groups.
- exp() is split: most tiles on ScalarE (exact spline), a fraction on
  VectorE via a double-Schraudolph bit-trick (two fp32->int16 affine
  converts bitcast as bf16, multiplied; ~2% ripple, scale cancels in
  softmax). This balances the two engines; ScalarE alone would be the
  bottleneck at ~210us.
- Softmax denominator rides the attention matmul as an extra all-ones
  v-column (row 64 of the 65-row accumulator). PSUM is evacuated with a
  single [65, QT] copy (frees the PSUM slot fast); denominators of a head
  pair round-trip through DRAM to pack 2x1024 values as [128, 16], one
  cheap approx-reciprocal, then DMA-broadcast per head for the normalize
  multiply (SBUF -> bf16 attn).
- PE is warmed with dummy matmuls from t~=0 so the HAM clock-gate opens
  before the projection phase instead of 50us into it.
"""

import numpy as np
import ml_dtypes

HIDDEN = 768
NUM_HEADS = 12
D = 64
B = 4
S = 2048
P = 128

H_CORE = 6          # heads per core
OC = H_CORE * D     # 384 output channels per core for q/k
WAUG = H_CORE * (D + 1)  # 390: v columns with interleaved ones-columns
C_CHUNKS = HIDDEN // P   # 6
QT = 1024           # q-tile (free dim) for the attention inner loop
MMF = 512           # matmul moving free dim
STILES = S // P     # 16
QTILES = OC // P    # 3
NQH = S // QT       # 2
NF = QT // MMF      # 2

# double-Schraudolph exp approximation constants (bf16 int16 bit trick)
EXP_A = float(np.log2(np.e) * 64.0)
EXP_B1 = float(127 * 128 - 38.3)
EXP_B2 = float(127 * 128 + 24.2)

_CACHE = {}


def _build(with_mask: bool):
    import concourse.bass as bass
    import concourse.tile as tile
    from concourse import bacc, mybir
    from contextlib import ExitStack

    f32 = mybir.dt.float32
    bf16 = mybir.dt.bfloat16
    i16 = mybir.dt.int16
    AF = mybir.ActivationFunctionType
    ALU = mybir.AluOpType

    nc = bacc.Bacc(
        "TRN2",
        target_bir_lowering=False,
        debug=False,
        enable_asserts=True,
        num_devices=8,
    )

    x_d = nc.dram_tensor("x", (HIDDEN, S), bf16, kind="ExternalInput").ap()
    wq_d = nc.dram_tensor("wqT", (HIDDEN, OC), bf16, kind="ExternalInput").ap()
    bq_d = nc.dram_tensor("bq", (OC,), f32, kind="ExternalInput").ap()
    wk_d = nc.dram_tensor("wkT", (HIDDEN, OC), bf16, kind="ExternalInput").ap()
    bk_d = nc.dram_tensor("bk", (OC,), f32, kind="ExternalInput").ap()
    wv_d = nc.dram_tensor("wvT", (HIDDEN, WAUG), bf16, kind="ExternalInput").ap()
    bvb_d = nc.dram_tensor("bvb", (P, WAUG), f32, kind="ExternalInput").ap()
    wo_d = nc.dram_tensor("woT", (OC, HIDDEN), bf16, kind="ExternalInput").ap()
    bo_d = nc.dram_tensor("bo", (HIDDEN,), f32, kind="ExternalInput").ap()
    if with_mask:
        em_d = nc.dram_tensor("expmask", (S, S), f32, kind="ExternalInput").ap()
    out_d = nc.dram_tensor("out", (HIDDEN, S), f32, kind="ExternalOutput").ap()

    x_r = x_d.rearrange("(t p) s -> p t s", p=P)
    wq_r = wq_d.rearrange("(t p) o -> p t o", p=P)
    wk_r = wk_d.rearrange("(t p) o -> p t o", p=P)
    wv_r = wv_d.rearrange("(t p) o -> p t o", p=P)
    wo_r = wo_d.rearrange("(t p) o -> p t o", p=P)
    bq_r = bq_d.rearrange("(t p) -> p t", p=P)
    bk_r = bk_d.rearrange("(t p) -> p t", p=P)
    bo_r = bo_d.rearrange("(t p) -> p t", p=P)
    out_r = out_d.rearrange("(t p) s -> p t s", p=P)

    with tile.TileContext(nc) as tc, ExitStack() as ctx:
        consts = ctx.enter_context(tc.tile_pool(name="consts", bufs=1))
        persist = ctx.enter_context(tc.tile_pool(name="persist", bufs=1))

        bq_t = consts.tile([P, QTILES], f32)
        nc.sync.dma_start(bq_t[:], bq_r)
        bk_t = consts.tile([P, QTILES], f32)
        nc.sync.dma_start(bk_t[:], bk_r)
        bo_t = consts.tile([P, C_CHUNKS], f32)
        nc.sync.dma_start(bo_t[:], bo_r)
        bvb_t = consts.tile([P, WAUG], f32)
        nc.sync.dma_start(bvb_t[:], bvb_d)
        wo_t = consts.tile([P, QTILES, HIDDEN], bf16)
        nc.sync.dma_start(wo_t[:], wo_r)

        q_t = persist.tile([P, QTILES, S], bf16)
        k_t = persist.tile([P, QTILES, S], bf16)
        v_t = persist.tile([P, STILES, WAUG], bf16)
        attn_t = persist.tile([P, QTILES, S], bf16)

        # ---------------- phase A: projections ----------------
        with (
            tc.tile_pool(name="phA", bufs=1) as phA,
            tc.tile_pool(name="psA", bufs=2, space="PSUM") as psA,
        ):
            # HAM warmup from t=0: PE idles during the input DMA, and cold
            # matmuls run at 1.2 GHz. A continuous dummy-matmul stream from
            # kernel start keeps the PE busy through the HAM window so the
            # real projections start at 2.4 GHz.
            wut = phA.tile([P, MMF], bf16, name="wu_src")
            nc.vector.memset(wut[:], 0.0)
            wu = psA.tile([P, MMF], f32, tag="Pv", name="warmA")
            for i in range(44):
                nc.tensor.matmul(
                    wu[:], wut[:, 0:P], wut[:], start=True, stop=True,
                )

            x_t = phA.tile([P, C_CHUNKS, S], bf16)
            wq_t = phA.tile([P, C_CHUNKS, OC], bf16)
            wk_t = phA.tile([P, C_CHUNKS, OC], bf16)
            wv_t = phA.tile([P, C_CHUNKS, WAUG], bf16)
            for c in range(C_CHUNKS):
                nc.sync.dma_start(wq_t[:, c, :], wq_r[:, c, :])
                # split x chunk across two queues to halve its latency
                nc.sync.dma_start(x_t[:, c, 0:S // 2], x_r[:, c, 0:S // 2])
                nc.sync.dma_start(x_t[:, c, S // 2:S], x_r[:, c, S // 2:S])
                nc.sync.dma_start(wk_t[:, c, :], wk_r[:, c, :])
                nc.sync.dma_start(wv_t[:, c, :], wv_r[:, c, :])

            # q, k projections: out[o_tile(128), s] = sum_c WT[c,o]^T x[c,s]
            for ot in range(QTILES):
                for di, (dst, w_sb, b_sb) in enumerate(
                    ((q_t, wq_t, bq_t), (k_t, wk_t, bk_t))
                ):
                    for half in range(S // QT):
                        ps = psA.tile([P, QT], f32, tag="Pq")
                        for c in range(C_CHUNKS):
                            for nf in range(NF):
                                nc.tensor.matmul(
                                    ps[:, nf * MMF:(nf + 1) * MMF],
                                    w_sb[:, c, ot * P:(ot + 1) * P],
                                    x_t[:, c, half * QT + nf * MMF:
                                        half * QT + (nf + 1) * MMF],
                                    start=(c == 0),
                                    stop=(c == C_CHUNKS - 1),
                                )
                        # alternate evacuation engine: DVE and ScalarE both
                        # read PSUM; splitting halves the critical path
                        if (ot * 4 + di * 2 + half) % 2 == 0:
                            nc.vector.tensor_scalar_add(
                                dst[:, ot, half * QT:(half + 1) * QT],
                                ps[:],
                                b_sb[:, ot:ot + 1],
                            )
                        else:
                            nc.scalar.activation(
                                dst[:, ot, half * QT:(half + 1) * QT],
                                ps[:],
                                AF.Identity,
                                bias=b_sb[:, ot:ot + 1],
                            )

            # vT projection: out[s_tile(128), 390] = sum_c x[c,s]^T WvT[c,:]
            for st in range(STILES):
                ps = psA.tile([P, WAUG], f32, tag="Pv")
                for c in range(C_CHUNKS):
                    nc.tensor.matmul(
                        ps[:],
                        x_t[:, c, st * P:(st + 1) * P],
                        wv_t[:, c, :],
                        start=(c == 0),
                        stop=(c == C_CHUNKS - 1),
                    )
                nc.vector.tensor_tensor(
                    v_t[:, st, :], ps[:], bvb_t[:], ALU.add
                )

        # ---------------- phase B: attention ----------------
        with (
            tc.tile_pool(name="phB", bufs=4) as phB,
            tc.tile_pool(name="psB", bufs=2, space="PSUM") as psB,
            tc.tile_pool(name="outp", bufs=2) as outp,
            tc.tile_pool(name="dscr", bufs=2, space="DRAM") as dscr,
        ):
            # HAM warmup: the proj->attention boundary idles the PE long
            # enough to clock-gate it to 1.2 GHz, and the attention phase's
            # short busy bursts never un-throttle it on their own. A ~6us
            # dense burst of dummy matmuls flips it back to 2.4 GHz; the
            # steady-state attention gaps then never re-throttle it.
            wu = psB.tile([P, QT], f32, tag="S", name="warmup")
            for i in range(16):
                nc.tensor.matmul(
                    wu[:, 0:MMF], wo_t[:, 0, 0:P], wo_t[:, 0, 0:MMF],
                    start=True, stop=True,
                )
            for qh in range(NQH):
                for hp in range(H_CORE // 2):
                    heads = (2 * hp, 2 * hp + 1)
                    accs = [
                        psB.tile([D + 1, QT], f32, tag="A", name=f"acc{i}")
                        for i in range(2)
                    ]
                    def emit_accs(cc, ets):
                        for hi, h in enumerate(heads):
                            for nf in range(NF):
                                nc.tensor.matmul(
                                    accs[hi][:, nf * MMF:(nf + 1) * MMF],
                                    v_t[:, cc, 65 * h:65 * h + 65],
                                    ets[hi][:, nf * MMF:(nf + 1) * MMF],
                                    start=(cc == 0),
                                    stop=(cc == STILES - 1),
                                )

                    # one-chunk software pipeline: acc(c-1) is emitted after
                    # sc(c), so the exp of chunk c-1 (ScalarE spline or the
                    # 3-op DVE chain) computes while the PE streams chunk c's
                    # scores instead of stalling the in-order PE queue.
                    pipe = None
                    for c in range(STILES):
                        scs = [
                            psB.tile([P, QT], f32, tag="S", name=f"sc{i}")
                            for i in range(2)
                        ]
                        # head-pair scores interleaved: rows 0-63 and 64-127
                        # of the PE run concurrently (explicit tile_position)
                        for nf in range(NF):
                            for hi in range(2):
                                pb = 64 * hi
                                nc.tensor.matmul(
                                    scs[hi][:, nf * MMF:(nf + 1) * MMF],
                                    k_t[pb:pb + D, hp, c * P:(c + 1) * P],
                                    q_t[pb:pb + D, hp,
                                        qh * QT + nf * MMF:
                                        qh * QT + (nf + 1) * MMF],
                                    start=True,
                                    stop=True,
                                    tile_position=(pb, 0),
                                )
                        etiles = []
                        for hi in range(2):
                            e = phB.tile([P, QT], bf16, tag="E")
                            # ~25% of tiles take the approximate DVE path to
                            # offload the ScalarE exp bottleneck
                            if hi == 1 and c % 2 == 0:
                                y1 = phB.tile([P, QT], i16, tag="Y")
                                y2 = phB.tile([P, QT], i16, tag="Y")
                                nc.vector.tensor_scalar(
                                    y1[:], scs[hi][:], EXP_A, EXP_B1,
                                    ALU.mult, ALU.add,
                                )
                                nc.vector.tensor_scalar(
                                    y2[:], scs[hi][:], EXP_A, EXP_B2,
                                    ALU.mult, ALU.add,
                                )
                                nc.vector.tensor_tensor(
                                    e[:], y1[:].bitcast(bf16),
                                    y2[:].bitcast(bf16), ALU.mult,
                                )
                            else:
                                nc.scalar.activation(e[:], scs[hi][:], AF.Exp)
                            if with_mask:
                                em = phB.tile([P, QT], f32, tag="M")
                                nc.sync.dma_start(
                                    em[:],
                                    em_d[c * P:(c + 1) * P,
                                         qh * QT:(qh + 1) * QT],
                                )
                                nc.vector.tensor_tensor(
                                    e[:], e[:], em[:], ALU.mult
                                )
                            etiles.append(e)
                        if pipe is not None:
                            emit_accs(c - 1, pipe)
                        pipe = etiles
                    emit_accs(STILES - 1, pipe)

                    # ---- per-pair softmax normalization ----
                    # single [65, QT] copy evacuates numerators + denominator
                    # (frees the PSUM slot); denominators round-trip through
                    # DRAM packed as [128, 16] for one cheap reciprocal.
                    us = []
                    scr = dscr.tile([2, QT], f32, name=f"scr{qh}_{hp}")
                    for hi, h in enumerate(heads):
                        u = phB.tile([D + 1, QT], f32, tag="U", bufs=5,
                                     name=f"u{qh}_{h}")
                        nc.vector.tensor_copy(u[:], accs[hi][:])
                        nc.sync.dma_start(scr[hi:hi + 1, :], u[D:D + 1, :])
                        us.append(u)
                    pk = phB.tile([P, 2 * QT // P], f32, tag="PK", bufs=2,
                                  name=f"pk{qh}_{hp}")
                    nc.sync.dma_start(
                        pk[:], scr.rearrange("a (p n) -> (a p) n", p=P // 2)
                    )
                    rec = phB.tile([P, 2 * QT // P], f32, tag="PK", bufs=2,
                                   name=f"rec{qh}_{hp}")
                    nc.vector.reciprocal_approx_fast(rec[:], pk[:])
                    scr2 = dscr.tile([2, QT], f32, name=f"scr2_{qh}_{hp}")
                    nc.sync.dma_start(
                        scr2.rearrange("a (p n) -> (a p) n", p=P // 2), rec[:]
                    )
                    for hi, h in enumerate(heads):
                        bc = phB.tile([D, QT], f32, tag="B")
                        nc.sync.dma_start(
                            bc[:], scr2[hi:hi + 1, :].to_broadcast((D, QT))
                        )
                        pb = 64 * hi
                        nc.vector.tensor_tensor(
                            attn_t[pb:pb + D, hp, qh * QT:(qh + 1) * QT],
                            us[hi][0:D, :],
                            bc[:],
                            ALU.mult,
                        )

            # ---------------- output projection (tail) ----------------
            # re-warm the PE after the final normalization-chain gap
            wu2 = psB.tile([P, QT], f32, tag="S", name="warmup2")
            for i in range(12):
                nc.tensor.matmul(
                    wu2[:, 0:MMF], wo_t[:, 0, 0:P], wo_t[:, 0, 0:MMF],
                    start=True, stop=True,
                )
            for qh in range(NQH):
                for ot in range(C_CHUNKS):
                    ps = psB.tile([P, QT], f32, tag="S")
                    for ct in range(QTILES):
                        for nf in range(NF):
                            nc.tensor.matmul(
                                ps[:, nf * MMF:(nf + 1) * MMF],
                                wo_t[:, ct, ot * P:(ot + 1) * P],
                                attn_t[:, ct, qh * QT + nf * MMF:
                                       qh * QT + (nf + 1) * MMF],
                                start=(ct == 0),
                                stop=(ct == QTILES - 1),
                            )
                    o_sb = outp.tile([P, QT], f32, tag="O")
                    if ot % 2 == 0:
                        nc.vector.tensor_scalar_add(
                            o_sb[:], ps[:], bo_t[:, ot:ot + 1]
                        )
                    else:
                        nc.scalar.activation(
                            o_sb[:], ps[:], AF.Identity,
                            bias=bo_t[:, ot:ot + 1],
                        )
                    nc.sync.dma_start(
                        out_r[:, ot, qh * QT:(qh + 1) * QT], o_sb[:]
                    )

    nc.compile()
    return nc


def _get_program(with_mask: bool):
    key = ("prog", with_mask)
    if key not in _CACHE:
        _CACHE[key] = _build(with_mask)
    return _CACHE[key]


def _prep_inputs(hidden_state, mask, Wq, bq, Wk, bk, Wv, bv, Wo, bo):
    """Build the 8 per-core input dicts (host-side shard + weight prep)."""
    f = np.float32
    bf = ml_dtypes.bfloat16
    scale = np.float32(D ** -0.5)
    with_mask = bool(np.any(mask))

    in_maps = []
    for b in range(B):
        x_b = np.asarray(hidden_state[b, :, 0, :], dtype=f).astype(bf)
        if with_mask:
            em_b = np.exp(mask[b, :, 0, :].astype(f))
        for g in range(2):
            rows = slice(OC * g, OC * (g + 1))
            wqT = (np.asarray(Wq[rows, :], dtype=f) * scale).T.astype(bf)
            bqs = np.ascontiguousarray(np.asarray(bq[rows], dtype=f) * scale)
            wkT = np.asarray(Wk[rows, :], dtype=f).T.astype(bf)
            bks = np.ascontiguousarray(bk[rows], dtype=f)
            # augmented v weights: col 65h+j = Wv row, col 65h+64 = 0 (bias 1)
            wvT = np.zeros((HIDDEN, WAUG), dtype=f)
            bvb = np.zeros((WAUG,), dtype=f)
            for h in range(H_CORE):
                wvT[:, 65 * h:65 * h + 64] = np.asarray(
                    Wv[OC * g + D * h:OC * g + D * h + D, :], dtype=f).T
                bvb[65 * h:65 * h + 64] = bv[OC * g + D * h:OC * g + D * h + D]
                bvb[65 * h + 64] = 1.0
            woT = np.asarray(Wo[:, rows], dtype=f).T.astype(bf)
            m = {
                "x": np.ascontiguousarray(x_b),
                "wqT": np.ascontiguousarray(wqT),
                "bq": bqs,
                "wkT": np.ascontiguousarray(wkT),
                "bk": bks,
                "wvT": wvT.astype(bf),
                "bvb": np.broadcast_to(bvb, (P, WAUG)).copy(),
                "woT": np.ascontiguousarray(woT),
                "bo": (np.asarray(bo, dtype=f) * np.float32(0.5)),
            }
            if with_mask:
                m["expmask"] = em_b
            in_maps.append(m)
    return in_maps, with_mask


def run(inputs: dict, trace: bool = False):
    """Run on 8 NeuronCores; returns (full_output, exec_time_ns_or_None)."""
    from concourse import bass_utils

    in_maps, with_mask = _prep_inputs(**inputs)
    nc = _get_program(with_mask)
    res = bass_utils.run_bass_kernel_spmd(
        nc, in_maps, core_ids=list(range(8)), trace=trace
    )
    out = np.empty((B, HIDDEN, 1, S), dtype=np.float32)
    for b in range(B):
        out[b, :, 0, :] = res.results[2 * b]["out"] + res.results[2 * b + 1]["out"]
    return out, res.exec_time_ns


def kernel(**inputs) -> np.ndarray:
    out, _ = run(inputs, trace=False)
    return out


# revision 8
# speedup vs baseline: 1.6167x; 1.5564x over previous
"""Trainium2 Bass kernel for nn_Attention (B=4, S=2048, HIDDEN=768, 12 heads).

Sharding: 8 cores = 4 batches x 2 head-groups (6 heads each). Projection
weights are sliced per head-group and pre-transposed on the host; the
1/sqrt(64) scale is folded into Wq. Each core computes a partial output
(its head-group's contribution through Wo, with bo/2 bias); the host sums
the two partials per batch.

v2 design notes:
- All projection operands in bf16 (FWL-eligible weight loads; 128-col
  stationaries). Attention matmuls bf16 with fp32 PSUM accumulate.
- Scores for a head pair are issued interleaved with explicit
  tile_position (0,0)/(64,0) so the two 64-contraction matmuls run
  concurrently in separate PE row-groups.
- exp() is split: most tiles on ScalarE (exact spline), a fraction on
  VectorE via a double-Schraudolph bit-trick (two fp32->int16 affine
  converts bitcast as bf16, multiplied; ~2% ripple, scale cancels in
  softmax). This balances the two engines; ScalarE alone would be the
  bottleneck at ~210us.
- Softmax denominator rides the attention matmul as an extra all-ones
  v-column (row 64 of the 65-row accumulator). PSUM is evacuated with a
  single [65, QT] copy (frees the PSUM slot fast); denominators of a head
  pair round-trip through DRAM to pack 2x1024 values as [128, 16], one
  cheap approx-reciprocal, then DMA-broadcast per head for the normalize
  multiply (SBUF -> bf16 attn).
- PE is warmed with dummy matmuls from t~=0 so the HAM clock-gate opens
  before the projection phase instead of 50us into it.
"""

import numpy as np
import ml_dtypes

HIDDEN = 768
NUM_HEADS = 12
D = 64
B = 4
S = 2048
P = 128

H_CORE = 6          # heads per core
OC = H_CORE * D     # 384 output channels per core for q/k
WAUG = H_CORE * (D + 1)  # 390: v columns with interleaved ones-columns
C_CHUNKS = HIDDEN // P   # 6
QT = 1024           # q-tile (free dim) for the attention inner loop
MMF = 512           # matmul moving free dim
STILES = S // P     # 16
QTILES = OC // P    # 3
NQH = S // QT       # 2
NF = QT // MMF      # 2

# double-Schraudolph exp approximation constants (bf16 int16 bit trick)
EXP_A = float(np.log2(np.e) * 64.0)
EXP_B1 = float(127 * 128 - 38.3)
EXP_B2 = float(127 * 128 + 24.2)
DVE_EXP = False   # offload ~25% of exp tiles to VectorE (approx)

_CACHE = {}


def _build(with_mask: bool):
    import concourse.bass as bass
    import concourse.tile as tile
    from concourse import bacc, mybir
    from contextlib import ExitStack

    f32 = mybir.dt.float32
    bf16 = mybir.dt.bfloat16
    i16 = mybir.dt.int16
    AF = mybir.ActivationFunctionType
    ALU = mybir.AluOpType

    nc = bacc.Bacc(
        "TRN2",
        target_bir_lowering=False,
        debug=False,
        enable_asserts=True,
        num_devices=8,
    )

    x_d = nc.dram_tensor("x", (HIDDEN, S), bf16, kind="ExternalInput").ap()
    wq_d = nc.dram_tensor("wqT", (HIDDEN, OC), bf16, kind="ExternalInput").ap()
    bq_d = nc.dram_tensor("bq", (OC,), f32, kind="ExternalInput").ap()
    wk_d = nc.dram_tensor("wkT", (HIDDEN, OC), bf16, kind="ExternalInput").ap()
    bk_d = nc.dram_tensor("bk", (OC,), f32, kind="ExternalInput").ap()
    wv_d = nc.dram_tensor("wvT", (HIDDEN, WAUG), bf16, kind="ExternalInput").ap()
    bvb_d = nc.dram_tensor("bvb", (P, WAUG), f32, kind="ExternalInput").ap()
    wo_d = nc.dram_tensor("woT", (OC, HIDDEN), bf16, kind="ExternalInput").ap()
    bo_d = nc.dram_tensor("bo", (HIDDEN,), f32, kind="ExternalInput").ap()
    if with_mask:
        em_d = nc.dram_tensor("expmask", (S, S), f32, kind="ExternalInput").ap()
    out_d = nc.dram_tensor("out", (HIDDEN, S), f32, kind="ExternalOutput").ap()

    x_r = x_d.rearrange("(t p) s -> p t s", p=P)
    wq_r = wq_d.rearrange("(t p) o -> p t o", p=P)
    wk_r = wk_d.rearrange("(t p) o -> p t o", p=P)
    wv_r = wv_d.rearrange("(t p) o -> p t o", p=P)
    wo_r = wo_d.rearrange("(t p) o -> p t o", p=P)
    bq_r = bq_d.rearrange("(t p) -> p t", p=P)
    bk_r = bk_d.rearrange("(t p) -> p t", p=P)
    bo_r = bo_d.rearrange("(t p) -> p t", p=P)
    out_r = out_d.rearrange("(t p) s -> p t s", p=P)

    with tile.TileContext(nc) as tc, ExitStack() as ctx:
        consts = ctx.enter_context(tc.tile_pool(name="consts", bufs=1))
        persist = ctx.enter_context(tc.tile_pool(name="persist", bufs=1))

        bq_t = consts.tile([P, QTILES], f32)
        nc.sync.dma_start(bq_t[:], bq_r)
        bk_t = consts.tile([P, QTILES], f32)
        nc.sync.dma_start(bk_t[:], bk_r)
        bo_t = consts.tile([P, C_CHUNKS], f32)
        nc.sync.dma_start(bo_t[:], bo_r)
        bvb_t = consts.tile([P, WAUG], f32)
        nc.sync.dma_start(bvb_t[:], bvb_d)
        wo_t = consts.tile([P, QTILES, HIDDEN], bf16)
        nc.sync.dma_start(wo_t[:], wo_r)

        q_t = persist.tile([P, QTILES, S], bf16)
        k_t = persist.tile([P, QTILES, S], bf16)
        v_t = persist.tile([P, STILES, WAUG], bf16)
        attn_t = persist.tile([P, QTILES, S], bf16)

        # ---------------- phase A: projections ----------------
        with (
            tc.tile_pool(name="phA", bufs=1) as phA,
            tc.tile_pool(name="psA", bufs=2, space="PSUM") as psA,
        ):
            # HAM warmup from t=0: PE idles during the input DMA, and cold
            # matmuls run at 1.2 GHz. A continuous dummy-matmul stream from
            # kernel start keeps the PE busy through the HAM window so the
            # real projections start at 2.4 GHz.
            wut = phA.tile([P, MMF], bf16, name="wu_src")
            nc.vector.memset(wut[:], 0.0)
            wu = psA.tile([P, MMF], f32, tag="Pv", name="warmA")
            for i in range(44):
                nc.tensor.matmul(
                    wu[:], wut[:, 0:P], wut[:], start=True, stop=True,
                )

            x_t = phA.tile([P, C_CHUNKS, S], bf16)
            wq_t = phA.tile([P, C_CHUNKS, OC], bf16)
            wk_t = phA.tile([P, C_CHUNKS, OC], bf16)
            wv_t = phA.tile([P, C_CHUNKS, WAUG], bf16)
            for c in range(C_CHUNKS):
                nc.sync.dma_start(wq_t[:, c, :], wq_r[:, c, :])
                # split x chunk across two queues to halve its latency
                nc.sync.dma_start(x_t[:, c, 0:S // 2], x_r[:, c, 0:S // 2])
                nc.sync.dma_start(x_t[:, c, S // 2:S], x_r[:, c, S // 2:S])
                nc.sync.dma_start(wk_t[:, c, :], wk_r[:, c, :])
                nc.sync.dma_start(wv_t[:, c, :], wv_r[:, c, :])

            # q, k projections: out[o_tile(128), s] = sum_c WT[c,o]^T x[c,s]
            for ot in range(QTILES):
                for di, (dst, w_sb, b_sb) in enumerate(
                    ((q_t, wq_t, bq_t), (k_t, wk_t, bk_t))
                ):
                    for half in range(S // QT):
                        ps = psA.tile([P, QT], f32, tag="Pq")
                        for c in range(C_CHUNKS):
                            for nf in range(NF):
                                nc.tensor.matmul(
                                    ps[:, nf * MMF:(nf + 1) * MMF],
                                    w_sb[:, c, ot * P:(ot + 1) * P],
                                    x_t[:, c, half * QT + nf * MMF:
                                        half * QT + (nf + 1) * MMF],
                                    start=(c == 0),
                                    stop=(c == C_CHUNKS - 1),
                                )
                        # alternate evacuation engine: DVE and ScalarE both
                        # read PSUM; splitting halves the critical path
                        if (ot * 4 + di * 2 + half) % 2 == 0:
                            nc.vector.tensor_scalar_add(
                                dst[:, ot, half * QT:(half + 1) * QT],
                                ps[:],
                                b_sb[:, ot:ot + 1],
                            )
                        else:
                            nc.scalar.activation(
                                dst[:, ot, half * QT:(half + 1) * QT],
                                ps[:],
                                AF.Identity,
                                bias=b_sb[:, ot:ot + 1],
                            )

            # vT projection: out[s_tile(128), 390] = sum_c x[c,s]^T WvT[c,:]
            for st in range(STILES):
                ps = psA.tile([P, WAUG], f32, tag="Pv")
                for c in range(C_CHUNKS):
                    nc.tensor.matmul(
                        ps[:],
                        x_t[:, c, st * P:(st + 1) * P],
                        wv_t[:, c, :],
                        start=(c == 0),
                        stop=(c == C_CHUNKS - 1),
                    )
                nc.vector.tensor_tensor(
                    v_t[:, st, :], ps[:], bvb_t[:], ALU.add
                )

        # ---------------- phase B: attention ----------------
        with (
            tc.tile_pool(name="phB", bufs=4) as phB,
            tc.tile_pool(name="psB", bufs=2, space="PSUM") as psB,
            tc.tile_pool(name="outp", bufs=2) as outp,
            tc.tile_pool(name="dscr", bufs=2, space="DRAM") as dscr,
        ):
            # HAM warmup: the proj->attention boundary idles the PE long
            # enough to clock-gate it to 1.2 GHz, and the attention phase's
            # short busy bursts never un-throttle it on their own. A ~6us
            # dense burst of dummy matmuls flips it back to 2.4 GHz; the
            # steady-state attention gaps then never re-throttle it.
            wu = psB.tile([P, QT], f32, tag="S", name="warmup")
            for i in range(16):
                nc.tensor.matmul(
                    wu[:, 0:MMF], wo_t[:, 0, 0:P], wo_t[:, 0, 0:MMF],
                    start=True, stop=True,
                )
            for qh in range(NQH):
                for hp in range(H_CORE // 2):
                    heads = (2 * hp, 2 * hp + 1)
                    accs = [
                        psB.tile([D + 1, QT], f32, tag="A", name=f"acc{i}")
                        for i in range(2)
                    ]
                    def emit_accs(cc, ets):
                        for hi, h in enumerate(heads):
                            for nf in range(NF):
                                nc.tensor.matmul(
                                    accs[hi][:, nf * MMF:(nf + 1) * MMF],
                                    v_t[:, cc, 65 * h:65 * h + 65],
                                    ets[hi][:, nf * MMF:(nf + 1) * MMF],
                                    start=(cc == 0),
                                    stop=(cc == STILES - 1),
                                )

                    # one-chunk software pipeline: acc(c-1) is emitted after
                    # sc(c), so the exp of chunk c-1 (ScalarE spline or the
                    # 3-op DVE chain) computes while the PE streams chunk c's
                    # scores instead of stalling the in-order PE queue.
                    pipe = None
                    for c in range(STILES):
                        scs = [
                            psB.tile([P, QT], f32, tag="S", name=f"sc{i}")
                            for i in range(2)
                        ]
                        # head-pair scores interleaved: rows 0-63 and 64-127
                        # of the PE run concurrently (explicit tile_position)
                        for nf in range(NF):
                            for hi in range(2):
                                pb = 64 * hi
                                nc.tensor.matmul(
                                    scs[hi][:, nf * MMF:(nf + 1) * MMF],
                                    k_t[pb:pb + D, hp, c * P:(c + 1) * P],
                                    q_t[pb:pb + D, hp,
                                        qh * QT + nf * MMF:
                                        qh * QT + (nf + 1) * MMF],
                                    start=True,
                                    stop=True,
                                    tile_position=(pb, 0),
                                )
                        etiles = []
                        for hi in range(2):
                            e = phB.tile([P, QT], bf16, tag="E")
                            # ~25% of tiles take the approximate DVE path to
                            # offload the ScalarE exp bottleneck
                            if DVE_EXP and hi == 1 and c % 2 == 0:
                                y1 = phB.tile([P, QT], i16, tag="Y")
                                y2 = phB.tile([P, QT], i16, tag="Y")
                                nc.vector.tensor_scalar(
                                    y1[:], scs[hi][:], EXP_A, EXP_B1,
                                    ALU.mult, ALU.add,
                                )
                                nc.vector.tensor_scalar(
                                    y2[:], scs[hi][:], EXP_A, EXP_B2,
                                    ALU.mult, ALU.add,
                                )
                                nc.vector.tensor_tensor(
                                    e[:], y1[:].bitcast(bf16),
                                    y2[:].bitcast(bf16), ALU.mult,
                                )
                            else:
                                nc.scalar.activation(e[:], scs[hi][:], AF.Exp)
                            if with_mask:
                                em = phB.tile([P, QT], f32, tag="M")
                                nc.sync.dma_start(
                                    em[:],
                                    em_d[c * P:(c + 1) * P,
                                         qh * QT:(qh + 1) * QT],
                                )
                                nc.vector.tensor_tensor(
                                    e[:], e[:], em[:], ALU.mult
                                )
                            etiles.append(e)
                        if pipe is not None:
                            emit_accs(c - 1, pipe)
                        pipe = etiles
                    emit_accs(STILES - 1, pipe)

                    # ---- per-pair softmax normalization ----
                    # single [65, QT] copy evacuates numerators + denominator
                    # (frees the PSUM slot); denominators round-trip through
                    # DRAM packed as [128, 16] for one cheap reciprocal.
                    us = []
                    scr = dscr.tile([2, QT], f32, name=f"scr{qh}_{hp}")
                    for hi, h in enumerate(heads):
                        u = phB.tile([D + 1, QT], f32, tag="U", bufs=5,
                                     name=f"u{qh}_{h}")
                        nc.vector.tensor_copy(u[:], accs[hi][:])
                        nc.sync.dma_start(scr[hi:hi + 1, :], u[D:D + 1, :])
                        us.append(u)
                    pk = phB.tile([P, 2 * QT // P], f32, tag="PK", bufs=2,
                                  name=f"pk{qh}_{hp}")
                    nc.sync.dma_start(
                        pk[:], scr.rearrange("a (p n) -> (a p) n", p=P // 2)
                    )
                    rec = phB.tile([P, 2 * QT // P], f32, tag="PK", bufs=2,
                                   name=f"rec{qh}_{hp}")
                    nc.vector.reciprocal_approx_fast(rec[:], pk[:])
                    scr2 = dscr.tile([2, QT], f32, name=f"scr2_{qh}_{hp}")
                    nc.sync.dma_start(
                        scr2.rearrange("a (p n) -> (a p) n", p=P // 2), rec[:]
                    )
                    for hi, h in enumerate(heads):
                        bc = phB.tile([D, QT], f32, tag="B")
                        nc.sync.dma_start(
                            bc[:], scr2[hi:hi + 1, :].to_broadcast((D, QT))
                        )
                        pb = 64 * hi
                        nc.vector.tensor_tensor(
                            attn_t[pb:pb + D, hp, qh * QT:(qh + 1) * QT],
                            us[hi][0:D, :],
                            bc[:],
                            ALU.mult,
                        )

            # ---------------- output projection (tail) ----------------
            # re-warm the PE after the final normalization-chain gap
            wu2 = psB.tile([P, QT], f32, tag="S", name="warmup2")
            for i in range(12):
                nc.tensor.matmul(
                    wu2[:, 0:MMF], wo_t[:, 0, 0:P], wo_t[:, 0, 0:MMF],
                    start=True, stop=True,
                )
            for qh in range(NQH):
                for ot in range(C_CHUNKS):
                    ps = psB.tile([P, QT], f32, tag="S")
                    for ct in range(QTILES):
                        for nf in range(NF):
                            nc.tensor.matmul(
                                ps[:, nf * MMF:(nf + 1) * MMF],
                                wo_t[:, ct, ot * P:(ot + 1) * P],
                                attn_t[:, ct, qh * QT + nf * MMF:
                                       qh * QT + (nf + 1) * MMF],
                                start=(ct == 0),
                                stop=(ct == QTILES - 1),
                            )
                    o_sb = outp.tile([P, QT], f32, tag="O")
                    if ot % 2 == 0:
                        nc.vector.tensor_scalar_add(
                            o_sb[:], ps[:], bo_t[:, ot:ot + 1]
                        )
                    else:
                        nc.scalar.activation(
                            o_sb[:], ps[:], AF.Identity,
                            bias=bo_t[:, ot:ot + 1],
                        )
                    nc.sync.dma_start(
                        out_r[:, ot, qh * QT:(qh + 1) * QT], o_sb[:]
                    )

    nc.compile()
    return nc


def _get_program(with_mask: bool):
    key = ("prog", with_mask)
    if key not in _CACHE:
        _CACHE[key] = _build(with_mask)
    return _CACHE[key]


def _prep_inputs(hidden_state, mask, Wq, bq, Wk, bk, Wv, bv, Wo, bo):
    """Build the 8 per-core input dicts (host-side shard + weight prep)."""
    f = np.float32
    bf = ml_dtypes.bfloat16
    scale = np.float32(D ** -0.5)
    with_mask = bool(np.any(mask))

    in_maps = []
    for b in range(B):
        x_b = np.asarray(hidden_state[b, :, 0, :], dtype=f).astype(bf)
        if with_mask:
            em_b = np.exp(mask[b, :, 0, :].astype(f))
        for g in range(2):
            rows = slice(OC * g, OC * (g + 1))
            wqT = (np.asarray(Wq[rows, :], dtype=f) * scale).T.astype(bf)
            bqs = np.ascontiguousarray(np.asarray(bq[rows], dtype=f) * scale)
            wkT = np.asarray(Wk[rows, :], dtype=f).T.astype(bf)
            bks = np.ascontiguousarray(bk[rows], dtype=f)
            # augmented v weights: col 65h+j = Wv row, col 65h+64 = 0 (bias 1)
            wvT = np.zeros((HIDDEN, WAUG), dtype=f)
            bvb = np.zeros((WAUG,), dtype=f)
            for h in range(H_CORE):
                wvT[:, 65 * h:65 * h + 64] = np.asarray(
                    Wv[OC * g + D * h:OC * g + D * h + D, :], dtype=f).T
                bvb[65 * h:65 * h + 64] = bv[OC * g + D * h:OC * g + D * h + D]
                bvb[65 * h + 64] = 1.0
            woT = np.asarray(Wo[:, rows], dtype=f).T.astype(bf)
            m = {
                "x": np.ascontiguousarray(x_b),
                "wqT": np.ascontiguousarray(wqT),
                "bq": bqs,
                "wkT": np.ascontiguousarray(wkT),
                "bk": bks,
                "wvT": wvT.astype(bf),
                "bvb": np.broadcast_to(bvb, (P, WAUG)).copy(),
                "woT": np.ascontiguousarray(woT),
                "bo": (np.asarray(bo, dtype=f) * np.float32(0.5)),
            }
            if with_mask:
                m["expmask"] = em_b
            in_maps.append(m)
    return in_maps, with_mask


def run(inputs: dict, trace: bool = False):
    """Run on 8 NeuronCores; returns (full_output, exec_time_ns_or_None)."""
    from concourse import bass_utils

    in_maps, with_mask = _prep_inputs(**inputs)
    nc = _get_program(with_mask)
    res = bass_utils.run_bass_kernel_spmd(
        nc, in_maps, core_ids=list(range(8)), trace=trace
    )
    out = np.empty((B, HIDDEN, 1, S), dtype=np.float32)
    for b in range(B):
        out[b, :, 0, :] = res.results[2 * b]["out"] + res.results[2 * b + 1]["out"]
    return out, res.exec_time_ns


def kernel(**inputs) -> np.ndarray:
    out, _ = run(inputs, trace=False)
    return out


# revision 10
# speedup vs baseline: 1.8716x; 1.1577x over previous
"""Trainium2 Bass kernel for nn_Attention (B=4, S=2048, HIDDEN=768, 12 heads).

Sharding: 8 cores = 4 batches x 2 head-groups (6 heads each). Projection
weights are sliced per head-group and pre-transposed on the host; the
1/sqrt(64) scale is folded into Wq. Each core computes a partial output
(its head-group's contribution through Wo, with bo/2 bias); the host sums
the two partials per batch.

v2 design notes:
- All projection operands in bf16 (FWL-eligible weight loads; 128-col
  stationaries). Attention matmuls bf16 with fp32 PSUM accumulate.
- Scores for a head pair are issued interleaved with explicit
  tile_position (0,0)/(64,0) so the two 64-contraction matmuls run
  concurrently in separate PE row-groups.
- exp() is split: most tiles on ScalarE (exact spline), a fraction on
  VectorE via a double-Schraudolph bit-trick (two fp32->int16 affine
  converts bitcast as bf16, multiplied; ~2% ripple, scale cancels in
  softmax). This balances the two engines; ScalarE alone would be the
  bottleneck at ~210us.
- Softmax denominator rides the attention matmul as an extra all-ones
  v-column (row 64 of the 65-row accumulator). PSUM is evacuated with a
  single [65, QT] copy (frees the PSUM slot fast); denominators of a head
  pair round-trip through DRAM to pack 2x1024 values as [128, 16], one
  cheap approx-reciprocal, then DMA-broadcast per head for the normalize
  multiply (SBUF -> bf16 attn).
- PE is warmed with dummy matmuls from t~=0 so the HAM clock-gate opens
  before the projection phase instead of 50us into it.
"""

import numpy as np
import ml_dtypes

HIDDEN = 768
NUM_HEADS = 12
D = 64
B = 4
S = 2048
P = 128

H_CORE = 6          # heads per core
OC = H_CORE * D     # 384 output channels per core for q/k
WAUG = H_CORE * (D + 1)  # 390: v columns with interleaved ones-columns
C_CHUNKS = HIDDEN // P   # 6
QT = 1024           # q-tile (free dim) for the attention inner loop
MMF = 512           # matmul moving free dim
STILES = S // P     # 16
QTILES = OC // P    # 3
NQH = S // QT       # 2
NF = QT // MMF      # 2

# single-pass Schraudolph exp approximation (fp32 -> int16 bitcast bf16):
# e = bitcast_bf16(int16(s * 128*log2(e) + 127*128 + C)), max rel err ~3.3%
# (pure mantissa-sawtooth ripple; the scale component cancels in softmax)
EXP_A = float(np.log2(np.e) * 128.0)
EXP_B = float(127 * 128 - 5.5)
DVE_EXP = True    # offload a fraction of exp tiles to VectorE (approx)

_CACHE = {}


def _build(with_mask: bool):
    import concourse.bass as bass
    import concourse.tile as tile
    from concourse import bacc, mybir
    from contextlib import ExitStack

    f32 = mybir.dt.float32
    bf16 = mybir.dt.bfloat16
    i16 = mybir.dt.int16
    AF = mybir.ActivationFunctionType
    ALU = mybir.AluOpType

    nc = bacc.Bacc(
        "TRN2",
        target_bir_lowering=False,
        debug=False,
        enable_asserts=True,
        num_devices=8,
    )

    x_d = nc.dram_tensor("x", (HIDDEN, S), bf16, kind="ExternalInput").ap()
    wq_d = nc.dram_tensor("wqT", (HIDDEN, OC), bf16, kind="ExternalInput").ap()
    bq_d = nc.dram_tensor("bq", (OC,), f32, kind="ExternalInput").ap()
    wk_d = nc.dram_tensor("wkT", (HIDDEN, OC), bf16, kind="ExternalInput").ap()
    bk_d = nc.dram_tensor("bk", (OC,), f32, kind="ExternalInput").ap()
    wv_d = nc.dram_tensor("wvT", (HIDDEN, WAUG), bf16, kind="ExternalInput").ap()
    bvb_d = nc.dram_tensor("bvb", (P, WAUG), f32, kind="ExternalInput").ap()
    wo_d = nc.dram_tensor("woT", (OC, HIDDEN), bf16, kind="ExternalInput").ap()
    bo_d = nc.dram_tensor("bo", (HIDDEN,), f32, kind="ExternalInput").ap()
    if with_mask:
        em_d = nc.dram_tensor("expmask", (S, S), f32, kind="ExternalInput").ap()
    out_d = nc.dram_tensor("out", (HIDDEN, S), f32, kind="ExternalOutput").ap()

    x_r = x_d.rearrange("(t p) s -> p t s", p=P)
    wq_r = wq_d.rearrange("(t p) o -> p t o", p=P)
    wk_r = wk_d.rearrange("(t p) o -> p t o", p=P)
    wv_r = wv_d.rearrange("(t p) o -> p t o", p=P)
    wo_r = wo_d.rearrange("(t p) o -> p t o", p=P)
    bq_r = bq_d.rearrange("(t p) -> p t", p=P)
    bk_r = bk_d.rearrange("(t p) -> p t", p=P)
    bo_r = bo_d.rearrange("(t p) -> p t", p=P)
    out_r = out_d.rearrange("(t p) s -> p t s", p=P)

    with tile.TileContext(nc) as tc, ExitStack() as ctx:
        consts = ctx.enter_context(tc.tile_pool(name="consts", bufs=1))
        persist = ctx.enter_context(tc.tile_pool(name="persist", bufs=1))

        bq_t = consts.tile([P, QTILES], f32)
        nc.sync.dma_start(bq_t[:], bq_r)
        bk_t = consts.tile([P, QTILES], f32)
        nc.sync.dma_start(bk_t[:], bk_r)
        bo_t = consts.tile([P, C_CHUNKS], f32)
        nc.sync.dma_start(bo_t[:], bo_r)
        bvb_t = consts.tile([P, WAUG], f32)
        nc.sync.dma_start(bvb_t[:], bvb_d)
        wo_t = consts.tile([P, QTILES, HIDDEN], bf16)
        nc.sync.dma_start(wo_t[:], wo_r)

        q_t = persist.tile([P, QTILES, S], bf16)
        k_t = persist.tile([P, QTILES, S], bf16)
        v_t = persist.tile([P, STILES, WAUG], bf16)
        attn_t = persist.tile([P, QTILES, S], bf16)

        # ---------------- phase A: projections ----------------
        with (
            tc.tile_pool(name="phA", bufs=1) as phA,
            tc.tile_pool(name="psA", bufs=2, space="PSUM") as psA,
        ):
            # HAM warmup from t=0: PE idles during the input DMA, and cold
            # matmuls run at 1.2 GHz. A continuous dummy-matmul stream from
            # kernel start keeps the PE busy through the HAM window so the
            # real projections start at 2.4 GHz.
            wut = phA.tile([P, MMF], bf16, name="wu_src")
            nc.vector.memset(wut[:], 0.0)
            wu = psA.tile([P, MMF], f32, tag="Pv", name="warmA")
            for i in range(44):
                nc.tensor.matmul(
                    wu[:], wut[:, 0:P], wut[:], start=True, stop=True,
                )

            x_t = phA.tile([P, C_CHUNKS, S], bf16)
            wq_t = phA.tile([P, C_CHUNKS, OC], bf16)
            wk_t = phA.tile([P, C_CHUNKS, OC], bf16)
            wv_t = phA.tile([P, C_CHUNKS, WAUG], bf16)
            for c in range(C_CHUNKS):
                nc.sync.dma_start(wq_t[:, c, :], wq_r[:, c, :])
                # split x chunk across two queues to halve its latency
                nc.sync.dma_start(x_t[:, c, 0:S // 2], x_r[:, c, 0:S // 2])
                nc.sync.dma_start(x_t[:, c, S // 2:S], x_r[:, c, S // 2:S])
                nc.sync.dma_start(wk_t[:, c, :], wk_r[:, c, :])
                nc.sync.dma_start(wv_t[:, c, :], wv_r[:, c, :])

            # q, k projections: out[o_tile(128), s] = sum_c WT[c,o]^T x[c,s]
            for ot in range(QTILES):
                for di, (dst, w_sb, b_sb) in enumerate(
                    ((q_t, wq_t, bq_t), (k_t, wk_t, bk_t))
                ):
                    for half in range(S // QT):
                        ps = psA.tile([P, QT], f32, tag="Pq")
                        for c in range(C_CHUNKS):
                            for nf in range(NF):
                                nc.tensor.matmul(
                                    ps[:, nf * MMF:(nf + 1) * MMF],
                                    w_sb[:, c, ot * P:(ot + 1) * P],
                                    x_t[:, c, half * QT + nf * MMF:
                                        half * QT + (nf + 1) * MMF],
                                    start=(c == 0),
                                    stop=(c == C_CHUNKS - 1),
                                )
                        # alternate evacuation engine: DVE and ScalarE both
                        # read PSUM; splitting halves the critical path
                        if (ot * 4 + di * 2 + half) % 2 == 0:
                            nc.vector.tensor_scalar_add(
                                dst[:, ot, half * QT:(half + 1) * QT],
                                ps[:],
                                b_sb[:, ot:ot + 1],
                            )
                        else:
                            nc.scalar.activation(
                                dst[:, ot, half * QT:(half + 1) * QT],
                                ps[:],
                                AF.Identity,
                                bias=b_sb[:, ot:ot + 1],
                            )

            # vT projection: out[s_tile(128), 390] = sum_c x[c,s]^T WvT[c,:]
            for st in range(STILES):
                ps = psA.tile([P, WAUG], f32, tag="Pv")
                for c in range(C_CHUNKS):
                    nc.tensor.matmul(
                        ps[:],
                        x_t[:, c, st * P:(st + 1) * P],
                        wv_t[:, c, :],
                        start=(c == 0),
                        stop=(c == C_CHUNKS - 1),
                    )
                nc.vector.tensor_tensor(
                    v_t[:, st, :], ps[:], bvb_t[:], ALU.add
                )

        # ---------------- phase B: attention ----------------
        with (
            tc.tile_pool(name="phB", bufs=4) as phB,
            tc.tile_pool(name="psB", bufs=2, space="PSUM") as psB,
            tc.tile_pool(name="outp", bufs=2) as outp,
            tc.tile_pool(name="dscr", bufs=2, space="DRAM") as dscr,
        ):
            # HAM warmup: the proj->attention boundary idles the PE long
            # enough to clock-gate it to 1.2 GHz, and the attention phase's
            # short busy bursts never un-throttle it on their own. A ~6us
            # dense burst of dummy matmuls flips it back to 2.4 GHz; the
            # steady-state attention gaps then never re-throttle it.
            wu = psB.tile([P, QT], f32, tag="S", name="warmup")
            for i in range(16):
                nc.tensor.matmul(
                    wu[:, 0:MMF], wo_t[:, 0, 0:P], wo_t[:, 0, 0:MMF],
                    start=True, stop=True,
                )
            for qh in range(NQH):
                for hp in range(H_CORE // 2):
                    heads = (2 * hp, 2 * hp + 1)
                    accs = [
                        psB.tile([D + 1, QT], f32, tag="A", name=f"acc{i}")
                        for i in range(2)
                    ]
                    def emit_accs(cc, ets):
                        for hi, h in enumerate(heads):
                            for nf in range(NF):
                                nc.tensor.matmul(
                                    accs[hi][:, nf * MMF:(nf + 1) * MMF],
                                    v_t[:, cc, 65 * h:65 * h + 65],
                                    ets[hi][:, nf * MMF:(nf + 1) * MMF],
                                    start=(cc == 0),
                                    stop=(cc == STILES - 1),
                                )

                    # one-chunk software pipeline: acc(c-1) is emitted after
                    # sc(c), so the exp of chunk c-1 (ScalarE spline or the
                    # 3-op DVE chain) computes while the PE streams chunk c's
                    # scores instead of stalling the in-order PE queue.
                    pipe = None
                    for c in range(STILES):
                        scs = [
                            psB.tile([P, QT], f32, tag="S", name=f"sc{i}")
                            for i in range(2)
                        ]
                        # head-pair scores interleaved: rows 0-63 and 64-127
                        # of the PE run concurrently (explicit tile_position)
                        for nf in range(NF):
                            for hi in range(2):
                                pb = 64 * hi
                                nc.tensor.matmul(
                                    scs[hi][:, nf * MMF:(nf + 1) * MMF],
                                    k_t[pb:pb + D, hp, c * P:(c + 1) * P],
                                    q_t[pb:pb + D, hp,
                                        qh * QT + nf * MMF:
                                        qh * QT + (nf + 1) * MMF],
                                    start=True,
                                    stop=True,
                                    tile_position=(pb, 0),
                                )
                        etiles = []
                        for hi in range(2):
                            e = phB.tile([P, QT], bf16, tag="E")
                            # ~25% of tiles take the approximate DVE path to
                            # offload the ScalarE exp bottleneck
                            if DVE_EXP and hi == 1 and c % 2 == 0:
                                nc.vector.tensor_scalar(
                                    e[:].bitcast(i16), scs[hi][:],
                                    EXP_A, EXP_B, ALU.mult, ALU.add,
                                )
                            else:
                                nc.scalar.activation(e[:], scs[hi][:], AF.Exp)
                            if with_mask:
                                em = phB.tile([P, QT], f32, tag="M")
                                nc.sync.dma_start(
                                    em[:],
                                    em_d[c * P:(c + 1) * P,
                                         qh * QT:(qh + 1) * QT],
                                )
                                nc.vector.tensor_tensor(
                                    e[:], e[:], em[:], ALU.mult
                                )
                            etiles.append(e)
                        if pipe is not None:
                            emit_accs(c - 1, pipe)
                        pipe = etiles
                    emit_accs(STILES - 1, pipe)

                    # ---- per-pair softmax normalization ----
                    # single [65, QT] copy evacuates numerators + denominator
                    # (frees the PSUM slot); denominators round-trip through
                    # DRAM packed as [128, 16] for one cheap reciprocal.
                    us = []
                    scr = dscr.tile([2, QT], f32, name=f"scr{qh}_{hp}")
                    for hi, h in enumerate(heads):
                        u = phB.tile([D + 1, QT], f32, tag="U", bufs=5,
                                     name=f"u{qh}_{h}")
                        nc.vector.tensor_copy(u[:], accs[hi][:])
                        nc.sync.dma_start(scr[hi:hi + 1, :], u[D:D + 1, :])
                        us.append(u)
                    pk = phB.tile([P, 2 * QT // P], f32, tag="PK", bufs=2,
                                  name=f"pk{qh}_{hp}")
                    nc.sync.dma_start(
                        pk[:], scr.rearrange("a (p n) -> (a p) n", p=P // 2)
                    )
                    rec = phB.tile([P, 2 * QT // P], f32, tag="PK", bufs=2,
                                   name=f"rec{qh}_{hp}")
                    nc.vector.reciprocal_approx_fast(rec[:], pk[:])
                    scr2 = dscr.tile([2, QT], f32, name=f"scr2_{qh}_{hp}")
                    nc.sync.dma_start(
                        scr2.rearrange("a (p n) -> (a p) n", p=P // 2), rec[:]
                    )
                    for hi, h in enumerate(heads):
                        bc = phB.tile([D, QT], f32, tag="B")
                        nc.sync.dma_start(
                            bc[:], scr2[hi:hi + 1, :].to_broadcast((D, QT))
                        )
                        pb = 64 * hi
                        nc.vector.tensor_tensor(
                            attn_t[pb:pb + D, hp, qh * QT:(qh + 1) * QT],
                            us[hi][0:D, :],
                            bc[:],
                            ALU.mult,
                        )

            # ---------------- output projection (tail) ----------------
            # re-warm the PE after the final normalization-chain gap
            wu2 = psB.tile([P, QT], f32, tag="S", name="warmup2")
            for i in range(12):
                nc.tensor.matmul(
                    wu2[:, 0:MMF], wo_t[:, 0, 0:P], wo_t[:, 0, 0:MMF],
                    start=True, stop=True,
                )
            for qh in range(NQH):
                for ot in range(C_CHUNKS):
                    ps = psB.tile([P, QT], f32, tag="S")
                    for ct in range(QTILES):
                        for nf in range(NF):
                            nc.tensor.matmul(
                                ps[:, nf * MMF:(nf + 1) * MMF],
                                wo_t[:, ct, ot * P:(ot + 1) * P],
                                attn_t[:, ct, qh * QT + nf * MMF:
                                       qh * QT + (nf + 1) * MMF],
                                start=(ct == 0),
                                stop=(ct == QTILES - 1),
                            )
                    o_sb = outp.tile([P, QT], f32, tag="O")
                    if ot % 2 == 0:
                        nc.vector.tensor_scalar_add(
                            o_sb[:], ps[:], bo_t[:, ot:ot + 1]
                        )
                    else:
                        nc.scalar.activation(
                            o_sb[:], ps[:], AF.Identity,
                            bias=bo_t[:, ot:ot + 1],
                        )
                    nc.sync.dma_start(
                        out_r[:, ot, qh * QT:(qh + 1) * QT], o_sb[:]
                    )

    nc.compile()
    return nc


def _get_program(with_mask: bool):
    key = ("prog", with_mask)
    if key not in _CACHE:
        _CACHE[key] = _build(with_mask)
    return _CACHE[key]


def _prep_inputs(hidden_state, mask, Wq, bq, Wk, bk, Wv, bv, Wo, bo):
    """Build the 8 per-core input dicts (host-side shard + weight prep)."""
    f = np.float32
    bf = ml_dtypes.bfloat16
    scale = np.float32(D ** -0.5)
    with_mask = bool(np.any(mask))

    in_maps = []
    for b in range(B):
        x_b = np.asarray(hidden_state[b, :, 0, :], dtype=f).astype(bf)
        if with_mask:
            em_b = np.exp(mask[b, :, 0, :].astype(f))
        for g in range(2):
            rows = slice(OC * g, OC * (g + 1))
            wqT = (np.asarray(Wq[rows, :], dtype=f) * scale).T.astype(bf)
            bqs = np.ascontiguousarray(np.asarray(bq[rows], dtype=f) * scale)
            wkT = np.asarray(Wk[rows, :], dtype=f).T.astype(bf)
            bks = np.ascontiguousarray(bk[rows], dtype=f)
            # augmented v weights: col 65h+j = Wv row, col 65h+64 = 0 (bias 1)
            wvT = np.zeros((HIDDEN, WAUG), dtype=f)
            bvb = np.zeros((WAUG,), dtype=f)
            for h in range(H_CORE):
                wvT[:, 65 * h:65 * h + 64] = np.asarray(
                    Wv[OC * g + D * h:OC * g + D * h + D, :], dtype=f).T
                bvb[65 * h:65 * h + 64] = bv[OC * g + D * h:OC * g + D * h + D]
                bvb[65 * h + 64] = 1.0
            woT = np.asarray(Wo[:, rows], dtype=f).T.astype(bf)
            m = {
                "x": np.ascontiguousarray(x_b),
                "wqT": np.ascontiguousarray(wqT),
                "bq": bqs,
                "wkT": np.ascontiguousarray(wkT),
                "bk": bks,
                "wvT": wvT.astype(bf),
                "bvb": np.broadcast_to(bvb, (P, WAUG)).copy(),
                "woT": np.ascontiguousarray(woT),
                "bo": (np.asarray(bo, dtype=f) * np.float32(0.5)),
            }
            if with_mask:
                m["expmask"] = em_b
            in_maps.append(m)
    return in_maps, with_mask


def run(inputs: dict, trace: bool = False):
    """Run on 8 NeuronCores; returns (full_output, exec_time_ns_or_None)."""
    from concourse import bass_utils

    in_maps, with_mask = _prep_inputs(**inputs)
    nc = _get_program(with_mask)
    res = bass_utils.run_bass_kernel_spmd(
        nc, in_maps, core_ids=list(range(8)), trace=trace
    )
    out = np.empty((B, HIDDEN, 1, S), dtype=np.float32)
    for b in range(B):
        out[b, :, 0, :] = res.results[2 * b]["out"] + res.results[2 * b + 1]["out"]
    return out, res.exec_time_ns


def kernel(**inputs) -> np.ndarray:
    out, _ = run(inputs, trace=False)
    return out
